# revision 73
# baseline (speedup 1.0000x reference)
"""Trainium2 Bass kernel for GCN(1->8) + flatten + big regression matvec.

Model (reference):
    h = GCNConv(x[4096,1], edge_index[2,131072], W1[1,8], b1[8])   # [4096, 8]
    h = relu(h.reshape(-1))                                        # [32768]
    y = h @ Wr[32768, 4096] + br                                   # [4096]

Since x is [N,1] and W1 is [1,8], the GCN collapses to a per-node scalar
    s[d] = dinv[d] * sum_src C'[d, src] * u[src],   u = x * dinv,
and h[d,k] = relu(s[d]*W1[k] + b1[k]).

Key idea: with b1 == 0 (the spec fill), relu(s*w_k) = s*w_k whenever
sign(w_k) == sign(s), else 0, so node d's total contribution to y is
    s_d * P_sel[d, :],   P_pos[d] = sum_{k: w_k>0} w_k * Wr[d,k,:],
                         P_neg[d] = sum_{k: w_k<0} w_k * Wr[d,k,:].
P_pos/P_neg are weight-only combinations of Wr rows (W1, Wr are module
weights), precomputed on the host (standard weight folding) and stored as
a stacked bf16 table.  Each node then dma_gathers exactly ONE 4096-wide
bf16 row -- the data-minimal HBM traffic -- and the matvec is
    y += s^T @ gathered_rows   (bf16 matmul into per-column psum banks).

All activation-dependent arithmetic (aggregation over edge_index,
normalization, s, row selection, matvec) runs on device; the host only
does graph/table layout, weight folding, and dtype casts.

The gather indices depend on the full aggregation, so the DMA window
between the end of the C' stream and the first gather (~4us: DMA-sem
propagation + index chain + SWDGE descriptor generation + trigger) is
covered by STATIC prefetches that need no index: both sign blocks of
chunk 0 (all columns) and chunk 1 (first SC1 columns), stored fp8 at
SCALE8 with the 1/SCALE8 folded into exact power-of-two bf16
coefficients (max(s,0)/SCALE8 picks P_pos, min(s,0)/SCALE8 picks
P_neg).  fp8 static costs the same bytes per covered column as the
bf16 gather it replaces, so the coverage is stream-neutral; its
quantization noise on ~17% of the nodes prices the whole kernel at
rel err ~1.5e-2 against the 2e-2 gate.

General b1 != 0 / br != 0 is handled exactly by the same structure:
the 8 lines relu(s*w_k + b_k) change their live set at breakpoints
t_k = -b_k/w_k; the R <= 9 s-intervals each get folded tables P_i
(s-coefficients) and Q_i (constants, br folded in as br/N per row),
and the node's interval index picks the gathered rows (two gathers
per node: P row with coeff s, Q row with coeff 1).

Sharding: row-parallel split across 8 cores (core k owns nodes
[512k, 512k+512) and their folded table rows).  Message passing is a
dense fp8 matmul against the core's [4096, 512] slice of C' with u split
into three scaled fp8 terms (fp32-accurate).  Each core emits a partial
y[4096]; the host sums the 8 partials in f64.

Schedule (per core, graded path): ct chunk 0 first, packed/x in the
first inter-chunk slot, rest of the C' stream (GCN matmuls interleave
per chunk; the last piece is a single source block so its matmuls
clear the 900ns DMA-sem quickly), idx constants, the fp8 static
prefetches sized to end exactly when the index chain can trigger the
first gather, then 8 column-split gathers ordered so psum banks retire
progressively: banks 0-2 after chunk 3's first h0 half, 3-5 after its
second, bank 6 one gather before the end, and only bank 7 rides the
final transfer.  Banks 6/7 accumulate as [2, 256] column quarters (via
zero/s routing lhsT variants) so their tail psum->sbuf copies run on
two partitions at half cost; y leaves in two DMAs (cols [0:HS) mid-
stream on the SP queue, [HS:Y) at the tail on the Act queue).
"""

import numpy as np
import ml_dtypes

import concourse.bacc as bacc
import concourse.bass as bass
import concourse.mybir as mybir
import concourse.tile as tile
from concourse.bass_utils import run_bass_kernel_spmd

N = 4096            # nodes
HID = 8             # GCN hidden dim
Y = 4096            # output dim
NCORES = 8
NPC = N // NCORES   # 512 nodes per core
SC1 = 1408          # chunk-1 fp8 static prefetch columns (R==2 only)
HS = 3072           # gather column split point
SCALE8 = 16.0       # fp8 static-table pre-scale (power of two)

F32 = mybir.dt.float32
FP8 = mybir.dt.float8e4
BF16 = mybir.dt.bfloat16
I32 = mybir.dt.int32
I16 = mybir.dt.int16
AF = mybir.ActivationFunctionType
OP = mybir.AluOpType

BF16_NP = ml_dtypes.bfloat16
FP8_NP = ml_dtypes.float8_e4m3


def _build_kernel(R=2, use_q=False, ct_bf16=False, taps=False):
    """R = number of s-interval table blocks (2 when b1 == 0).
    use_q: gather constant-term Q rows too (b1 != 0 or br != 0; br rides
    inside the Q tables as a br/N-per-row weight fold)."""
    CW = 32                       # idx cols: 4 chunks x 8 stripes
    static_ok = (R == 2) and not use_q
    NT = 1 if ct_bf16 else 3      # u terms
    u_dt = BF16 if ct_bf16 else FP8

    nc = bacc.Bacc("TRN2", target_bir_lowering=False, debug=False,
                   num_devices=NCORES)

    pk_d = nc.dram_tensor("packed", [128, 96], I32, kind="ExternalInput")
    # idx consts (f32): 0:CW = K8A (stripe mask * 512), CW:2CW = C0 base
    ix_d = nc.dram_tensor("idxconsts", [128, 2 * CW], F32,
                          kind="ExternalInput")
    le_d = nc.dram_tensor("lefold", [128, 128], BF16, kind="ExternalInput")
    ct_dt = BF16 if ct_bf16 else FP8
    ct_d = nc.dram_tensor("ct", [N, NPC], ct_dt, kind="ExternalInput")
    # thresholds for the interval index (replicated across partitions; only
    # read when R > 2 -- for R == 2 the single threshold is 0)
    th_d = nc.dram_tensor("thresh", [128, max(R - 1, 1)], F32,
                          kind="ExternalInput")
    nrows = R * NPC * (2 if use_q else 1)
    wrp_d = nc.dram_tensor("wrp", [nrows, Y], BF16, kind="ExternalInput")
    # chunk 0's and chunk 1's P rows (both blocks) as a SCALE8-scaled fp8
    # static table: the byte cost per covered column equals the bf16 gather
    # it replaces, so this coverage is stream-neutral and needs no gather
    # index; rows = [Pp_c0; Pn_c0; Pp_c1; Pn_c1]
    wr8_d = nc.dram_tensor("wrp8", [512, Y], FP8, kind="ExternalInput")
    y_d = nc.dram_tensor("y", [1, Y], F32, kind="ExternalOutput")
    if taps:
        tap_d = nc.dram_tensor("tap", [128, 16], F32, kind="ExternalOutput")
        tapidx_d = nc.dram_tensor("tapidx", [128, 32], F32,
                                  kind="ExternalOutput")

    with tile.TileContext(nc) as tc:
        with (
            tc.tile_pool(name="small", bufs=1) as sp,
            tc.tile_pool(name="wr", bufs=1) as wp_pool,
            tc.tile_pool(name="psum", bufs=1, space="PSUM") as pp,
        ):
            # ---- DMA issue order (SP queue): ct0 first (longest-lead
            # stream), packed in the first inter-chunk slot, ct1-3, idx
            # consts, static prefetch, bias ----
            pk_sb = sp.tile([128, 96], I32)
            x_sb = pk_sb[:, 0:32].bitcast(F32)
            inda_sb = pk_sb[:, 32:64]
            indb_sb = pk_sb[:, 64:96]
            # ct in 5 pieces (the last chunk split in half) so most of the
            # final chunk's matmuls clear before the last piece's 900ns DMA
            # semaphore propagation
            ct_pieces = [(0, 8), (8, 8), (16, 8), (24, 7), (31, 1)]
            ct_tiles = []
            for pi_, (sc0, nsc) in enumerate(ct_pieces):
                ctc = sp.tile([128, nsc * NPC], ct_dt, name=f"ct{pi_}")
                ct_tiles.append(ctc)
                nc.sync.dma_start(
                    out=ctc[:].rearrange("p (sc q) -> p sc q", q=NPC),
                    in_=ct_d[128 * sc0:128 * (sc0 + nsc), :].rearrange(
                        "(sc p) q -> p sc q", p=128))
                if pi_ == 0:
                    nc.sync.dma_start(out=pk_sb[:], in_=pk_d[:])
            ix_sb = sp.tile([128, 2 * CW], F32)
            k8a_sb = ix_sb[:, 0:CW]
            c0_sb = ix_sb[:, CW:2 * CW]
            nc.sync.dma_start(out=ix_sb[:], in_=ix_d[:])
            le_sb = sp.tile([128, 128], BF16)
            nc.sync.dma_start(out=le_sb[:], in_=le_d[:])
            th_sb = sp.tile([128, max(R - 1, 1)], F32)
            if R > 2:
                nc.sync.dma_start(out=th_sb[:], in_=th_d[:])
            st8_tiles = []
            st1_tiles = []
            if static_ok:
                for blk in range(2):
                    t = sp.tile([128, Y], FP8, name=f"st8{blk}")
                    st8_tiles.append(t)
                    nc.sync.dma_start(out=t[:], in_=wr8_d[128 * blk:
                                                          128 * (blk + 1), :])
                for blk in range(2):
                    t = sp.tile([128, SC1], FP8, name=f"st1{blk}")
                    st1_tiles.append(t)
                    nc.sync.dma_start(
                        out=t[:], in_=wr8_d[256 + 128 * blk:
                                            256 + 128 * (blk + 1), 0:SC1])
            # ---- term-scale const tile, stored t-major so memsets are
            # contiguous; viewed (db, t) when multiplying the psum ----
            scl_sb = sp.tile([128, 4 * NT], F32)
            scales = (1.0,) if ct_bf16 else (1.0, 1.0 / 64, 1.0 / 4096)
            for ti, v in enumerate(scales):
                nc.vector.memset(scl_sb[:, 4 * ti:4 * ti + 4], v)
            # psum accumulator zeroed up front; matmuls then accumulate with
            # start=False so the four interleaved dst-block groups sharing
            # this bank never reset each other (start=True zeroes the whole
            # 2KB bank region)
            agg_ps = pp.tile([128, 4 * NT], F32, name="ps0")
            nc.vector.memset(agg_ps[:], 0.0)
            ones_sb = sp.tile([128, 4], BF16)
            if use_q:
                nc.vector.memset(ones_sb[:], 1.0)

            # ---- deg -> dinv (exact: Sqrt + bit-exact reciprocal) ----
            degf_sb = sp.tile([128, 32], F32)
            degi_sb = sp.tile([128, 32], I32)
            nc.vector.tensor_tensor(out=degi_sb[:], in0=indb_sb,
                                    in1=inda_sb, op=OP.subtract)
            nc.vector.tensor_scalar_add(degi_sb[:], degi_sb[:], 1)
            nc.vector.tensor_copy(out=degf_sb[:], in_=degi_sb[:])
            sq_sb = sp.tile([128, 32], F32)
            nc.scalar.activation(sq_sb[:], degf_sb[:], AF.Sqrt)
            dinv_sb = sp.tile([128, 32], F32)
            nc.vector.reciprocal(dinv_sb[:], sq_sb[:])

            # ---- u = x*dinv, split into NT scaled terms ----
            u_sb = sp.tile([128, 32], F32)
            nc.vector.tensor_tensor(out=u_sb[:], in0=x_sb, in1=dinv_sb[:],
                                    op=OP.mult)
            u2_sb = sp.tile([128, 32 * NT], u_dt)
            u2v = u2_sb[:].rearrange("p (c t) -> p c t", t=NT)
            if ct_bf16:
                nc.vector.tensor_copy(out=u2_sb[:], in_=u_sb[:])
            else:
                res_sb = sp.tile([128, 32], F32)
                for term, scale in enumerate((1.0, 64.0, 4096.0)):
                    scl2_sb = sp.tile([128, 32], F32, name=f"scl{term}")
                    if scale == 1.0:
                        src_ap = u_sb[:]
                    else:
                        nc.vector.tensor_scalar_mul(
                            scl2_sb[:], u_sb[:] if term == 0 else res_sb[:],
                            scale)
                        src_ap = scl2_sb[:]
                    nc.vector.tensor_copy(
                        out=u2v[:, :, term:term + 1],
                        in_=src_ap.rearrange("p (c one) -> p c one", one=1))
                    if term < 2:
                        back_sb = sp.tile([128, 32], F32, name=f"back{term}")
                        nc.vector.tensor_copy(
                            out=back_sb[:].rearrange("p (c one) -> p c one",
                                                     one=1),
                            in_=u2v[:, :, term:term + 1])
                        if scale != 1.0:
                            nc.vector.tensor_scalar_mul(back_sb[:], back_sb[:],
                                                        1.0 / scale)
                        nc.vector.tensor_tensor(
                            out=res_sb[:],
                            in0=(u_sb[:] if term == 0 else res_sb[:]),
                            in1=back_sb[:], op=OP.subtract)

            # ---- agg[d] = sum_src C'[d, src] * u[src] ----
            for pi_, (sc0, nsc) in enumerate(ct_pieces):
                ctc = ct_tiles[pi_]
                for sci in range(nsc):
                    sc = sc0 + sci
                    base = NPC * sci
                    for db in range(4):
                        nc.tensor.matmul(
                            out=agg_ps[:, NT * db:NT * (db + 1)],
                            lhsT=ctc[:, base + 128 * db:
                                     base + 128 * (db + 1)],
                            rhs=u2_sb[:, NT * sc:NT * sc + NT],
                            start=False, stop=(sc == 31),
                            skip_group_check=True)

            # ---- agg: scale terms + reduce; s = agg * dinv_own ----
            agg_sb = sp.tile([128, 4], F32)
            if NT == 1:
                nc.vector.tensor_copy(out=agg_sb[:], in_=agg_ps[:])
            else:
                aggt_sb = sp.tile([128, 4 * NT], F32)
                av = aggt_sb[:].rearrange("p (db t) -> p db t", t=NT)
                nc.vector.tensor_tensor(
                    out=av,
                    in0=agg_ps[:].rearrange("p (db t) -> p db t", t=NT),
                    in1=scl_sb[:].rearrange("p (t db) -> p db t", db=4),
                    op=OP.mult)
                nc.vector.tensor_reduce(out=agg_sb[:], in_=av,
                                        axis=mybir.AxisListType.X, op=OP.add)
            # ---- interval selector (f32) ----
            iv_sb = sp.tile([128, 4], F32)
            if R == 2:
                # sign(agg) == sign(s); key off agg, on DVE right after the
                # reduce (no cross-engine hop on the idx critical path)
                nc.vector.tensor_scalar(out=iv_sb[:], in0=agg_sb[:],
                                        scalar1=0.0, scalar2=None,
                                        op0=OP.is_le)
            else:
                sf_sb = sp.tile([128, 4], F32)
                nc.vector.tensor_tensor(out=sf_sb[:], in0=agg_sb[:],
                                        in1=dinv_sb[:, 0:4], op=OP.mult)
                tmp_sb = sp.tile([128, 4], F32)
                for j in range(R - 1):
                    # thresholds are input-dependent, so they ride in as a
                    # partition-replicated tile used as per-partition scalars
                    nc.gpsimd.tensor_scalar(
                        out=(iv_sb[:] if j == 0 else tmp_sb[:]),
                        in0=sf_sb[:], scalar1=th_sb[:, j:j + 1],
                        scalar2=None, op0=OP.is_le)
                    if j > 0:
                        nc.gpsimd.tensor_tensor(out=iv_sb[:], in0=iv_sb[:],
                                                in1=tmp_sb[:], op=OP.add)

            # ---- static-path masked coefficients (R == 2 only) ----
            if static_ok:
                shi_sb = sp.tile([128, 4], BF16)
                slo_sb = sp.tile([128, 4], BF16)

            # ---- gather idx: fold+replicate via LE matmul ----
            # nw[p, 8c+a] = iv[p, c] * K8A[p, 8c+a]  (K8A = stripe mask*512)
            nw_sb = sp.tile([128, CW], BF16)
            for c in range(4):
                eng = nc.vector if c % 2 == 0 else nc.gpsimd
                eng.tensor_scalar(out=nw_sb[:, 8 * c:8 * c + 8],
                                  in0=k8a_sb[:, 8 * c:8 * c + 8],
                                  scalar1=iv_sb[:, c:c + 1], scalar2=None,
                                  op0=OP.mult)
            idr_ps = pp.tile([128, CW], F32, name="ps1")
            nc.tensor.matmul(out=idr_ps[:], lhsT=le_sb[:], rhs=nw_sb[:],
                             start=True, stop=True)
            idx_sb = sp.tile([128, CW], I16)
            nc.vector.tensor_tensor(out=idx_sb[:], in0=idr_ps[:],
                                    in1=c0_sb, op=OP.add)
            # s and its derived coefficient tiles ride the DVE queue after
            # the idx chain (not needed until the first y matmuls)
            s_sb = sp.tile([128, 4], BF16)
            nc.vector.tensor_tensor(out=s_sb[:], in0=agg_sb[:],
                                    in1=dinv_sb[:, 0:4], op=OP.mult)
            if static_ok:
                nc.vector.tensor_scalar_max(shi_sb[:], s_sb[:], 0.0)
                nc.vector.tensor_tensor(out=slo_sb[:], in0=s_sb[:],
                                        in1=shi_sb[:], op=OP.subtract)
                # 1/SCALE8-folded chunk-0 coefficients (exact: power of 2)
                s8 = {}
                for nm, src in (("hi", shi_sb), ("lo", slo_sb)):
                    t8 = sp.tile([128, 4], BF16, name=f"s8{nm}")
                    nc.vector.tensor_scalar_mul(t8[:], src[:], 1.0 / SCALE8)
                    s8[nm] = t8

            # ---- y psum accumulators; banks 6 and 7 are [2, 256] (column
            # quarters on two partitions) so their tail psum->sbuf copies
            # run at half cost.  They are memset-zeroed and accumulated
            # with start=False (two 1KB tiles may share a bank; start=True
            # would zero the neighbor). ----
            y_tiles = [pp.tile([1, 512], F32, name=f"ps{bk}")
                       for bk in range(6)]
            y_ps = [t[:] for t in y_tiles]
            y67 = [pp.tile([2, 256], F32, name=f"ps{bk}") for bk in (6, 7)]
            nc.vector.memset(y67[0][:], 0.0)
            nc.vector.memset(y67[1][:], 0.0)
            # lhsT variants routing a chunk's s column to psum partition 0/1
            szr = [sp.tile([128, 8], BF16, name=f"sz{r}") for r in (0, 1)]
            for r in (0, 1):
                nc.vector.memset(szr[r][:], 0.0)
            for r in (0, 1):
                nc.vector.tensor_copy(
                    out=szr[r][:].rearrange("p (c two) -> p c two",
                                            two=2)[:, :, r:r + 1],
                    in_=s_sb[:].rearrange("p (c one) -> p c one", one=1))
            if static_ok:
                # chunk-0 fp8-static coefficient variants for banks 6/7
                s8r = {}
                for nm in ("hi", "lo"):
                    for r in (0, 1):
                        t8 = sp.tile([128, 2], BF16, name=f"s8r{nm}{r}")
                        nc.vector.memset(t8[:], 0.0)
                        nc.vector.tensor_copy(out=t8[:, r:r + 1],
                                              in_=s8[nm][:, 0:1])
                        s8r[(nm, r)] = t8
            if use_q:
                ozr = [sp.tile([128, 8], BF16, name=f"oz{r}") for r in (0, 1)]
                for r in (0, 1):
                    nc.vector.memset(ozr[r][:], 0.0)
                    nc.vector.memset(
                        ozr[r][:].rearrange("p (c two) -> p c two",
                                            two=2)[:, :, r:r + 1], 1.0)

            first = [True] * 6 + [False, False]  # start-flag pending
            stops = [0] * 8              # emitted mm count per bank

            # gather plan: (chunk, col_lo, col_hi); h0 halves then h1.  A
            # full-width gather goes first so its transfer time covers the
            # next descriptor generation (no desc-gen pipeline gap).  The
            # last two chunks' h1 pieces are split per bank so bank 6
            # closes out early and only bank 7 rides the final transfer.
            # Chunk 0 is entirely static (fp8); chunk 1's first SC1 columns
            # are bf16-static.
            if static_ok:
                # chunk 3's h0 half is split so banks 0-2 stop (and copy
                # out) one gather earlier; c2's h1 stays whole to hold the
                # gather count at 8 (Pool desc-gen rate limits the tail)
                plan = [(2, 0, HS), (1, SC1, HS), (3, 0, 1536),
                        (3, 1536, HS), (1, HS, Y), (2, HS, Y),
                        (3, HS, HS + 512), (3, HS + 512, Y)]
            else:
                plan = [(1, 0, HS), (0, 0, HS), (2, 0, HS), (3, 0, HS),
                        (0, HS, Y), (1, HS, Y),
                        (2, HS, HS + 512), (3, HS, HS + 512),
                        (2, HS + 512, Y), (3, HS + 512, Y)]
            def bank_spans(lo, hi):
                """(bank, col_lo, col_hi) pieces of [lo, hi), split at bank
                (512) boundaries for banks 0-5 and quarter (256) boundaries
                for banks 6/7."""
                out = []
                pos = lo
                while pos < hi:
                    bk = pos // 512
                    step = 256 if bk >= 6 else 512
                    nxt = min(hi, (pos // step + 1) * step)
                    out.append((bk, pos, nxt))
                    pos = nxt
                return out

            # count total mms per bank to set stop on the last one
            total = [0] * 8
            if static_ok:
                for bk, _, _ in bank_spans(0, Y):     # chunk-0 fp8 static
                    total[bk] += 2
                for bk, _, _ in bank_spans(0, SC1):   # chunk-1 bf16 static
                    total[bk] += 2
            for c, lo, hi in plan:
                for bk, _, _ in bank_spans(lo, hi):
                    total[bk] += 2 if use_q else 1

            def mm(bk, cl, ch, lhs_col, rhs_ap, rvar=None):
                if bk < 6:
                    out_ap = y_ps[bk][:, cl - 512 * bk:ch - 512 * bk]
                    lhs = lhs_col
                    start = first[bk]
                else:
                    r = (cl // 256) % 2
                    out_ap = y67[bk - 6][:, cl - 512 * bk - 256 * r:
                                         ch - 512 * bk - 256 * r]
                    lhs = rvar[r]
                    start = False
                nonlast = stops[bk] + 1 < total[bk]
                nc.tensor.matmul(out=out_ap, lhsT=lhs, rhs=rhs_ap,
                                 start=start, stop=not nonlast,
                                 skip_group_check=True)
                first[bk] = False
                stops[bk] += 1

            # static matmuls: chunk 0 (fp8, all columns) then chunk 1
            # (fp8, cols [0:SC1)), both sign blocks each
            if static_ok:
                for bk, cl, ch in bank_spans(0, Y):
                    for nm, t in (("hi", st8_tiles[0]), ("lo", st8_tiles[1])):
                        mm(bk, cl, ch, s8[nm][:, 0:1], t[:, cl:ch],
                           rvar=[s8r[(nm, r)][:] for r in (0, 1)])
                for bk, cl, ch in bank_spans(0, SC1):
                    for nm, t in (("hi", st1_tiles[0]), ("lo", st1_tiles[1])):
                        mm(bk, cl, ch, s8[nm][:, 1:2], t[:, cl:ch])

            for c, lo, hi in plan:
                w = hi - lo
                srcs = [(wrp_d[0:R * NPC, lo:hi], s_sb, szr)]
                if use_q:
                    srcs.append((wrp_d[R * NPC:2 * R * NPC, lo:hi],
                                 ones_sb, ozr))
                for si, (src_ap, coef, rt) in enumerate(srcs):
                    t = wp_pool.tile([128, 1, w], BF16, name=f"g{c}_{lo}_{si}")
                    nc.gpsimd.dma_gather(
                        t[:], src_ap, idx_sb[:, 8 * c:8 * c + 8],
                        128, 128, w, elem_step=Y)
                    for bk, cl, ch in bank_spans(lo, hi):
                        mm(bk, cl, ch, coef[:, c:c + 1], t[:, 0, cl - lo:
                                                           ch - lo],
                           rvar=[rt[r][:, 2 * c:2 * c + 2] for r in (0, 1)])

            if taps:
                tap_sb = sp.tile([128, 16], F32)
                nc.vector.tensor_copy(out=tap_sb[:, 0:4], in_=s_sb[:])
                nc.vector.tensor_copy(out=tap_sb[:, 4:8], in_=iv_sb[:])
                nc.vector.tensor_copy(out=tap_sb[:, 8:12], in_=agg_sb[:])
                nc.vector.tensor_copy(out=tap_sb[:, 12:16],
                                      in_=dinv_sb[:, 0:4])
                nc.sync.dma_start(out=tap_d[:], in_=tap_sb[:])
                tapi_sb = sp.tile([128, 32], F32)
                nc.vector.tensor_copy(out=tapi_sb[:], in_=idx_sb[:])
                nc.sync.dma_start(out=tapidx_d[:], in_=tapi_sb[:])

            # ---- psum -> sbuf -> DRAM; banks 0-5 retire during the h1
            # stream; banks 6/7 copy as [2, 256] (two partitions in
            # parallel, half the serial cost) into one [2, 512] tile that
            # leaves via a single Act-queue dispatch ----
            y_sb = sp.tile([1, Y], F32)
            for bk in range(6):
                eng = (nc.vector.tensor_copy if bk % 2 == 0
                       else (lambda out, in_: nc.scalar.copy(out=out,
                                                             in_=in_)))
                eng(out=y_sb[:, 512 * bk:512 * (bk + 1)], in_=y_ps[bk])
                if bk == HS // 512 - 1:
                    nc.sync.dma_start(out=y_d[:, 0:HS], in_=y_sb[:, 0:HS])
            y67_sb = sp.tile([2, 512], F32)
            nc.vector.tensor_copy(out=y67_sb[:, 0:256], in_=y67[0][:])
            nc.scalar.copy(out=y67_sb[:, 256:512], in_=y67[1][:])
            nc.scalar.dma_start(
                out=y_d[:, HS:Y].rearrange("one (b r j) -> one r b j",
                                           b=2, r=2),
                in_=y67_sb[:].rearrange("r (b j) -> r b j", b=2))

    nc.compile()
    return nc


_NC_CACHE = {}


def _get_nc(R=2, use_q=False, ct_bf16=False):
    key = (R, use_q, ct_bf16)
    if key not in _NC_CACHE:
        _NC_CACHE[key] = _build_kernel(*key)
    return _NC_CACHE[key]


def _intervals(w, bv):
    """Sorted breakpoints (descending block order) and per-block live sets.

    Block i = live set of the i-th interval counting from s = +inf down;
    iv(d) = #breakpoints >= s_d selects the block."""
    brk = sorted({-bv[k] / w[k] for k in range(HID) if w[k] != 0})
    R = len(brk) + 1
    live = []
    for i in range(R):
        # representative point strictly inside interval i from the top
        if i == 0:
            sr = (brk[-1] + 1.0) if brk else 1.0
        elif i == R - 1:
            sr = brk[0] - 1.0
        else:
            sr = 0.5 * (brk[R - 2 - i] + brk[R - 1 - i])
        live.append([k for k in range(HID)
                     if (w[k] != 0 and w[k] * sr + bv[k] > 0)
                     or (w[k] == 0 and bv[k] > 0)])
    return brk, live


def _host_prep(x, edge_index, W1, b1, Wr, br):
    """Graph/table layout + weight folding + dtype casts; all
    activation-dependent FP arithmetic runs on device."""
    x = np.ascontiguousarray(x, dtype=np.float32).reshape(N)
    src = np.asarray(edge_index[0], dtype=np.int64)
    dst = np.asarray(edge_index[1], dtype=np.int64)

    indeg = np.bincount(dst, minlength=N)
    indptr = np.zeros(N + 1, dtype=np.int32)
    np.cumsum(indeg, out=indptr[1:])

    w = np.ascontiguousarray(W1, dtype=np.float32).reshape(HID)
    bv = np.ascontiguousarray(b1, dtype=np.float32).reshape(HID)
    brv = np.ascontiguousarray(br, dtype=np.float32).reshape(1, Y)
    Wr3 = np.ascontiguousarray(Wr, dtype=np.float32).reshape(N, HID, Y)

    brk, live = _intervals(w, bv)
    R = len(brk) + 1
    use_q = bool(np.any(bv != 0)) or bool(np.any(brv != 0))

    # interval thresholds, descending so iv = sum_j is_le(s, brk_desc[j]);
    # replicated across partitions for per-partition-scalar use
    th = np.zeros((128, max(R - 1, 1)), np.float32)
    th[:, :R - 1] = np.array(sorted(brk, reverse=True), np.float32)[None, :]

    # K8A[p, 8c+a] = (p//16 == a) * NPC ; C0[p, 8c+a] = 128c + 16a + p%16
    p_i = np.arange(128)[:, None]
    k8a = np.zeros((128, 32), np.float32)
    c0m = np.zeros((128, 32), np.float32)
    for c in range(4):
        for a in range(8):
            col = 8 * c + a
            k8a[:, col:col + 1] = (p_i // 16 == a) * float(NPC)
            c0m[:, col:col + 1] = 128 * c + 16 * a + p_i % 16
    le = np.tile((p_i % 16 == np.arange(16)[None, :]),
                 (1, 8)).astype(BF16_NP)

    in_maps = []
    p = np.arange(128)[:, None]
    ct_bf16_any = False
    for k in range(NCORES):
        rot = (np.arange(32) + 4 * k) % 32          # column rotation
        g = 128 * rot[None, :] + p                  # [128, 32] global node ids

        # dense count matrix for this core's dst rows, + I (self loops)
        mask = (dst >= NPC * k) & (dst < NPC * (k + 1))
        ck = np.zeros((NPC, N), dtype=np.float32)
        np.add.at(ck, (dst[mask] - NPC * k, src[mask]), 1.0)
        ck[np.arange(NPC), NPC * k + np.arange(NPC)] += 1.0
        ct_bf16 = bool(ck.max() > 8)
        ct_bf16_any |= ct_bf16
        ct_np = BF16_NP if ct_bf16 else FP8_NP
        srcperm = g.T.reshape(-1)                   # [(sc i)] -> global node
        ct = np.ascontiguousarray(ck[:, srcperm].T).astype(ct_np)

        # folded tables: P_i = sum_{k in live_i} w_k * Wr-rows (+ Q_i)
        Wk = Wr3[NPC * k:NPC * (k + 1)]             # [512, HID, Y]
        nrows = R * NPC * (2 if use_q else 1)
        wrp = np.zeros((nrows, Y), np.float32)
        for i in range(R):
            for kk in live[i]:
                wrp[i * NPC:(i + 1) * NPC] += w[kk] * Wk[:, kk, :]
                if use_q:
                    wrp[(R + i) * NPC:(R + i + 1) * NPC] += (
                        bv[kk] * Wk[:, kk, :])
            if use_q:
                # br folded as a per-row constant: the N gathered Q rows
                # across all cores sum to exactly br
                wrp[(R + i) * NPC:(R + i + 1) * NPC] += brv / N

        packed = np.concatenate([
            x[g].astype(np.float32).view(np.int32),
            indptr[g].astype(np.int32),
            indptr[g + 1].astype(np.int32)], axis=1)
        if R == 2 and not use_q:
            wrp8 = (np.concatenate([wrp[0:128], wrp[NPC:NPC + 128],
                                    wrp[128:256], wrp[NPC + 128:NPC + 256]])
                    * SCALE8).astype(FP8_NP)
        else:
            wrp8 = np.zeros((512, Y), dtype=FP8_NP)
        in_maps.append({
            "packed": np.ascontiguousarray(packed),
            "idxconsts": np.ascontiguousarray(
                np.concatenate([k8a, c0m], axis=1)),
            "lefold": le,
            "ct": ct,
            "thresh": th,
            "wrp": wrp.astype(BF16_NP),
            "wrp8": wrp8,
        })
    return in_maps, (R, use_q, ct_bf16_any)


def kernel(x, edge_index, W1, b1, Wr, br, _trace=False):
    in_maps, key = _host_prep(x, edge_index, W1, b1, Wr, br)
    nc = _get_nc(*key)
    try:
        res = run_bass_kernel_spmd(nc, in_maps, list(range(NCORES)),
                                   trace=_trace)
    except Exception:
        # one retry: recovers from transiently-poisoned device state
        res = run_bass_kernel_spmd(nc, in_maps, list(range(NCORES)),
                                   trace=_trace)
    y = np.zeros(Y, dtype=np.float64)
    for k in range(NCORES):
        y += np.asarray(res.results[k]["y"]).reshape(Y).astype(np.float64)
    out = y.astype(np.float32)
    if _trace:
        return out, res
    return out
